# revision 30
# baseline (speedup 1.0000x reference)
"""Trainium2 Bass kernel for masked-softmax attention scoring (v3).

Reference computation (B=128, T=512, K=1024, Q=1024):
    mids  = einsum("kq,bq->bk", W, query)
    s     = tanh(einsum("btk,bk->bt", key, mids) + bias)
    attn  = softmax-like: exp(s - max) * mask / sum(exp(s - max) * mask)

The max-subtraction cancels exactly in the ratio (tanh is bounded), so the
device computes  attn = exp(tanh(.)) * mask / sum_t(exp(tanh(.)) * mask).

Design (evolved from the v1 DVE kernel via trace analysis):
- Everything 16-bit on the wire: key/W/query cast to fp16 on the host
  (rel_l2 ~1.4e-3 vs the 2e-2 gate), halving the dominant HBM stream.
- Score dot-products run on the TensorEngine (not the DVE, whose fused
  mul-reduce is 1.13 us per 1024-col column): the host pre-transposes key
  to [pair, kc-pair, k-partition, kcsub, (b0 t | b1 t)] so the PE
  contracts over k on partitions.  1 MB tiles (4 matmuls each) keep the
  per-dma_start sequencer cost (~630 ns) off the critical path; a single
  sync-queue ring fans out across all 16 hardware DMA queues.
- PSUM accumulation groups must each own a full bank (interleaved groups
  at different byte offsets within one bank accumulate incorrectly):
  mids^T uses 8 banks (kc -> bank kc, qc-outer so matmuls overlap W's
  arrival); scores then reuse 4 of those banks, one per in-flight batch
  (bank = 2*(pr%2) + h, WAR distance = one full pair).
- The otherwise-idle Scalar engine extracts each batch row from PSUM row 0
  with a fused Tanh, then an Exp whose accum_out yields the raw row sum;
  pad slots contribute exactly exp(tanh(bias)) each, so the DVE just
  subtracts a host-computed pad total, takes the reciprocal, and scales
  -- no mask multiply.  All of it pipelines behind the PE stream.
- Mask compaction: masked-out timesteps (~14%) are never shipped; the
  host gathers kept t's per batch, the device computes only Tc = max
  kept-count columns, and the host scatters rows back (pad columns are
  discarded, so their garbage values are harmless).  The NEFF is
  compiled for the actual Tc on first call.

Sharding: data-parallel over B across 8 NeuronCores (16 batches/core).
"""

import sys

if "/opt/trn_rl_repo" not in sys.path:
    sys.path.insert(0, "/opt/trn_rl_repo")

from contextlib import ExitStack

import numpy as np

# ---- problem constants (hardcoded per spec) ----
B, T, K, Q = 128, 512, 1024, 1024
NCORES = 8
BS = B // NCORES          # 16 batches per core
P = 128                   # SBUF partitions
QC = Q // P               # 8 contraction chunks for the mids matmul
KC = K // P               # 8 contraction chunks for the scores matmul
KCP = KC // 2             # kc pairs per key tile
PR = BS // 2              # 8 batch pairs per core (2 batches per key tile)
KEY_BUFS = 14             # key tile pool depth (~3.5 KB/partition each)

_STATE: dict = {}


def _build_nc(Tc):
    import concourse.tile as tile
    from concourse import bacc, mybir

    f32 = mybir.dt.float32
    f16 = mybir.dt.float16
    nc = bacc.Bacc()

    qt_e = nc.declare_dram_parameter("qt", [P, QC, BS], f16, isOutput=False)
    wt_e = nc.declare_dram_parameter("wt", [P, KC, QC, P], f16, isOutput=False)
    keyt_e = nc.declare_dram_parameter(
        "keyt", [PR, KCP, P, 2, 2 * Tc], f16, isOutput=False
    )
    padc_e = nc.declare_dram_parameter("padc", [1, BS], f32, isOutput=False)
    bias_e = nc.declare_dram_parameter("biasb", [P, 1], f32, isOutput=False)
    out_e = nc.declare_dram_parameter("out", [BS, Tc], f32, isOutput=True)

    with tile.TileContext(nc) as tc, ExitStack() as ctx:
        const = ctx.enter_context(tc.tile_pool(name="const", bufs=1))
        kpool = ctx.enter_context(tc.tile_pool(name="key", bufs=KEY_BUFS))
        psum = ctx.enter_context(tc.tile_pool(name="psum", bufs=1, space="PSUM"))

        # 8 full psum banks: mids kc-group kc lives in pb[kc][:, :BS];
        # scores then reuse pb[0..3] row 0
        pb = [psum.tile([P, 512], f32, name=f"pb{i}") for i in range(8)]

        # ---- prologue loads ----
        # W streams kc-major: mids group kc only needs its own 256 KB
        # slice.  Slice 0 leads the sync ring (ahead of even qt) and the
        # rest ride the scalar ring, so the sync ring reaches the first
        # key tile almost immediately.
        wt_sb = const.tile([P, KC, QC, P], f16)
        nc.sync.dma_start(out=wt_sb[:, 0, :, :], in_=wt_e[:, 0, :, :])
        qt_sb = const.tile([P, QC, BS], f16)
        nc.sync.dma_start(out=qt_sb[:], in_=qt_e[:])
        for kc in range(1, KC):
            nc.scalar.dma_start(out=wt_sb[:, kc, :, :], in_=wt_e[:, kc, :, :])
        padc_sb = const.tile([1, BS], f32)
        nc.scalar.dma_start(out=padc_sb[:], in_=padc_e[:])
        bias_sb = const.tile([P, 1], f32)
        nc.scalar.dma_start(out=bias_sb[:], in_=bias_e[:])

        # ---- midsT[p, kc, b] = mids[b, kc*128+p] ----
        # kc-outer groups (one open accumulation group per bank, banks 4-7
        # rotating), with each group's midsT slice copied right after its
        # stop.  Groups for kc pair X are emitted just before the score
        # matmuls that need them, so mids work fills PE gaps in the
        # DMA-paced stream instead of serializing ahead of it.
        midsT_sb = const.tile([P, KC, BS], f16)

        def mids_group(kc):
            bank = 4 + kc % 4
            for qc in range(QC):
                nc.tensor.matmul(
                    pb[bank][:, :BS],
                    lhsT=wt_sb[:, kc, qc, :],
                    rhs=qt_sb[:, qc, :],
                    start=(qc == 0),
                    stop=(qc == QC - 1),
                )
            nc.vector.tensor_copy(midsT_sb[:, kc, :], pb[bank][:, :BS])

        # per-batch partition-0 tiles for the pipelined epilogue
        tanh_t = [const.tile([1, Tc], f32, name=f"tanh{b}") for b in range(BS)]
        exp_t = [const.tile([1, Tc], f32, name=f"exp{b}") for b in range(BS)]
        rsum_t = [const.tile([1, 1], f32, name=f"rsum{b}") for b in range(BS)]
        rinv_t = [const.tile([1, 1], f32, name=f"rinv{b}") for b in range(BS)]

        # ---- stream 1 MB key tiles on the sync ring; 4 matmuls per tile ----
        # During pair 0 the mids groups ladder into the score matmuls
        # (g0, tile0-j0, g1, tile0-j1, g2, ...) so streaming starts as
        # soon as group 0 lands instead of after the whole mids phase.
        for pr in range(PR):
            for kcp in range(KCP):
                kt = kpool.tile([P, 2, 2 * Tc], f16, tag="kt")
                nc.sync.dma_start(out=kt[:], in_=keyt_e[pr, kcp, :, :, :])
                if pr == 0:
                    for j in range(2):
                        kc = 2 * kcp + j
                        mids_group(kc)
                        for h in range(2):
                            bank = 2 * (pr % 2) + h
                            nc.tensor.matmul(
                                pb[bank][0:1, :Tc],
                                lhsT=midsT_sb[:, kc, h : h + 1],
                                rhs=kt[:, j, h * Tc : (h + 1) * Tc],
                                start=(kc == 0),
                                stop=(kc == KC - 1),
                            )
                else:
                    for h in range(2):
                        b = 2 * pr + h
                        bank = 2 * (pr % 2) + h
                        for j in range(2):
                            kc = 2 * kcp + j
                            nc.tensor.matmul(
                                pb[bank][0:1, :Tc],
                                lhsT=midsT_sb[:, kc, b : b + 1],
                                rhs=kt[:, j, h * Tc : (h + 1) * Tc],
                                start=(kc == 0),
                                stop=(kc == KC - 1),
                            )
            # pipelined per-batch epilogue: scalar does fused extract+tanh
            # from PSUM then exp; vector does mask+rowsum, reciprocal,
            # scale; gpsimd (software DGE) DMAs the finished row out.
            # Buffer reuse: masked-exp overwrites tanh_t, attn overwrites
            # exp_t.
            for h in range(2):
                b = 2 * pr + h
                bank = 2 * (pr % 2) + h
                nc.scalar.activation(
                    out=tanh_t[b][:],
                    in_=pb[bank][0:1, :Tc],
                    func=mybir.ActivationFunctionType.Tanh,
                    bias=bias_sb[0:1, :],
                    scale=1.0,
                )
                nc.scalar.activation(
                    out=exp_t[b][:],
                    in_=tanh_t[b][:],
                    func=mybir.ActivationFunctionType.Exp,
                    accum_out=rsum_t[b][:],
                )
                # pad slots contribute exactly exp(tanh(0+bias)) each to the
                # raw sum; subtract the host-computed total.  Pad columns of
                # the output row are garbage and discarded by the host.
                nc.vector.tensor_tensor(
                    out=rsum_t[b][:],
                    in0=rsum_t[b][:],
                    in1=padc_sb[:, b : b + 1],
                    op=mybir.AluOpType.subtract,
                )
                nc.vector.reciprocal(out=rinv_t[b][:], in_=rsum_t[b][:])
                nc.vector.tensor_scalar_mul(exp_t[b][:], exp_t[b][:], rinv_t[b][:])
            for h in range(2):
                b = 2 * pr + h
                nc.scalar.dma_start(out=out_e[b : b + 1, :], in_=exp_t[b][:])

    nc.compile()
    return nc


def _get_nc(Tc):
    if _STATE.get("Tc") != Tc:
        _STATE["nc"] = _build_nc(Tc)
        _STATE["Tc"] = Tc
    return _STATE["nc"]


def _make_in_maps(query, key, mask, W, bias):
    query = np.asarray(query, dtype=np.float32)
    key = np.asarray(key, dtype=np.float32)
    mask = np.asarray(mask, dtype=np.float32)
    W = np.asarray(W, dtype=np.float32)
    bias = np.asarray(bias, dtype=np.float32).reshape(-1)

    kept = [np.flatnonzero(mask[b] > 0.5) for b in range(B)]
    ns = np.array([len(k) for k in kept])
    Tc = int(ns.max())  # exact max kept count; all shapes compile from it
    if Tc > T:
        Tc = T
    _STATE["kept"] = kept
    _STATE["ns"] = ns
    _STATE["cur_Tc"] = Tc

    # wt[p, kc, qc, m] = W[kc*128+m, qc*128+p]  (kc-major; shared)
    WT = np.ascontiguousarray(
        np.ascontiguousarray(W.T).reshape(QC, P, KC, P).transpose(1, 2, 0, 3)
    ).astype(np.float16)
    biasb = np.ascontiguousarray(
        np.broadcast_to(bias[:1][None, :], (P, 1)).astype(np.float32)
    )

    pad_unit = float(np.exp(np.tanh(bias[0])))
    in_maps = []
    for i in range(NCORES):
        padc_row = np.ascontiguousarray(
            ((Tc - ns[i * BS : (i + 1) * BS]) * pad_unit).astype(np.float32)
        ).reshape(1, BS)
        sh = slice(i * BS, (i + 1) * BS)
        qt = np.ascontiguousarray(
            query[sh].T.reshape(QC, P, BS).transpose(1, 0, 2)
        ).astype(np.float16)
        kk = key[sh]
        kg = np.zeros((BS, Tc, K), dtype=np.float16)
        for bb in range(BS):
            kb = kept[i * BS + bb]
            kg[bb, : len(kb)] = kk[bb, kb]
        # keyt[pr, kcp, p, j, h*Tc + t] = kg[2*pr+h, t, (2*kcp+j)*128+p]
        keyt = np.ascontiguousarray(
            kg.reshape(PR, 2, Tc, KCP, 2, P).transpose(0, 3, 5, 4, 1, 2)
        )
        in_maps.append(
            {
                "qt": qt,
                "wt": WT,
                "keyt": keyt,
                "padc": padc_row,
                "biasb": biasb,
            }
        )
    return in_maps


def _run(in_maps, **kwargs):
    from concourse.bass_utils import run_bass_kernel_spmd

    return run_bass_kernel_spmd(
        _get_nc(_STATE["cur_Tc"]), in_maps, core_ids=list(range(NCORES)), **kwargs
    )


def _gather(results):
    out = np.zeros((B, T), dtype=np.float32)
    kept = _STATE["kept"]
    ns = _STATE["ns"]
    for i in range(NCORES):
        rows = np.asarray(results[i]["out"]).reshape(BS, _STATE["cur_Tc"])
        for bb in range(BS):
            b = i * BS + bb
            out[b, kept[b][: ns[b]]] = rows[bb, : ns[b]]
    return out


def kernel(query, key, mask, W, bias):
    in_maps = _make_in_maps(query, key, mask, W, bias)
    res = _run(in_maps)
    return _gather(res.results)
